# revision 47
# baseline (speedup 1.0000x reference)
"""Trainium2 Bass kernel for nn_DynamicConv (dense_cnn).

out[i, j, co, h, w] = sum_k (conv_k(x_i)[co, h, w] + b_k[co]) * attn[j, k]
attn = softmax(softmax(MLP(meanpool(x)), k) / TAU, k)

Sharding: data-parallel over batch i across 8 cores.  Each core convolves its
own sample (9 shifted bf16 matmuls over a zero-padded image, contraction =
CIN=128) and computes the full [B, K] attention matrix locally: every core
loads all 8 x-slices (bf16), mean-pools them on DVE, and runs the tiny MLP +
double softmax itself — no collective at all.

The cross-batch blend is a block-diagonal bf16 matmul per 16-channel group
(contraction 64 in one partition half, M = 128 = j8 x co16).  The two halves
(u=0 partitions 0-63, u=1 partitions 64-127) are emitted as ADJACENT matmuls:
they land on disjoint PE row-group strips (tile_position (0,0) / (64,0)) and
execute CONCURRENTLY.  One pair is interleaved after each conv row-group so
PSUM evictions never gate the PE: the pair's two banks drain on DVE+ACT well
within the next row-group's ~2us of conv matmuls.

Startup: xi is loaded in 5 row-chunks so the first conv matmul only waits for
~130KB + wt0; a short burst of dummy matmuls during the DMA wait warms the PE
HAM clock-gate so the real stream starts at 2.4 GHz.  Output slabs are stored
in halves as soon as their chunks evict, spread across the gpsimd and sync
DMA queues, shrinking the end-of-kernel DMA drain.

All matmul operands are bf16 (PE full rate); PSUM accumulates fp32; the
output slab is stored bf16 and widened to fp32 on the host.
"""

import sys

import numpy as np

if "/opt/trn_rl_repo" not in sys.path:
    sys.path.insert(0, "/opt/trn_rl_repo")

import ml_dtypes

import concourse.bacc as bacc
import concourse.bass as bass
import concourse.mybir as mybir
import concourse.tile as tile

F32 = mybir.dt.float32
BF16 = mybir.dt.bfloat16
AF = mybir.ActivationFunctionType
AX = mybir.AxisListType
ALU = mybir.AluOpType

B = 8
CIN = 128
COUT = 256
K = 4
KS = 3
HW = 48
HW2 = HW * HW          # 2304
WP = HW + 2            # 50 (padded)
HID = 256
TAU = 30.0
NCORES = 8

ROW_GROUPS = [(0, 10), (10, 10), (20, 10), (30, 10), (40, 8)]
CHUNKS = [(0, 512), (512, 512), (1024, 512), (1536, 512), (2048, 256)]
# xi row-chunk boundaries: chunk c covers exactly what conv row-group c needs
XCHUNKS = [(0, 11), (11, 21), (21, 31), (31, 41), (41, 48)]


def build_nc():
    nc = bacc.Bacc("TRN2", debug=False, num_devices=NCORES)

    xi = nc.dram_tensor("xi", [CIN, HW2], BF16, kind="ExternalInput").ap()
    xall = nc.dram_tensor("xall", [CIN, B * HW2], BF16, kind="ExternalInput").ap()
    # [ci, t, tap, p] flattened; p = c*4 + k encodes (co = 32 t + c, k)
    wconv = nc.dram_tensor(
        "wconv", [CIN, 8 * 9 * 128], BF16, kind="ExternalInput"
    ).ap()
    bconv = nc.dram_tensor("bconv", [128, 8], F32, kind="ExternalInput").ap()
    w1t = nc.dram_tensor("w1t", [CIN, HID], BF16, kind="ExternalInput").ap()
    b1c = nc.dram_tensor("b1c", [128, 2], F32, kind="ExternalInput").ap()
    w2t = nc.dram_tensor("w2t", [128, 2 * K], BF16, kind="ExternalInput").ap()
    b2r = nc.dram_tensor("b2r", [1, K], BF16, kind="ExternalInput").ap()
    one18 = nc.dram_tensor("one18", [1, B], BF16, kind="ExternalInput").ap()
    # constant selectors for the DMA-free BD2 build:
    #   jrep[j, col] = (col // 16 == j);  kexp[k, p] = (p % 4 == k)
    #   dmask[p, col] = (col % 16 == (p // 4) % 16)
    jrep = nc.dram_tensor("jrep", [B, 128], BF16, kind="ExternalInput").ap()
    kexp = nc.dram_tensor("kexp", [K, 128], BF16, kind="ExternalInput").ap()
    dmask = nc.dram_tensor("dmask", [128, 128], BF16, kind="ExternalInput").ap()
    out = nc.dram_tensor("out", [B, COUT, HW2], BF16, kind="ExternalOutput").ap()

    with tile.TileContext(nc, num_cores=NCORES) as tc:
        with (
            tc.tile_pool(name="const", bufs=1) as const,
            tc.tile_pool(name="csb", bufs=8) as csb_pool,
            tc.tile_pool(name="osb", bufs=8) as osb_pool,
            tc.tile_pool(name="psA", bufs=3, space="PSUM") as psA,
            tc.tile_pool(name="psB", bufs=4, space="PSUM") as psB,
            tc.tile_pool(name="psM", bufs=1, space="PSUM") as psM,
        ):
            # pre-warm the ACT function tables (1.3us each if loaded lazily
            # inside the latency-critical chains)
            zc = const.tile([128, 1], F32)
            nc.gpsimd.memset(zc[:], 0.0)
            actw = const.tile([128, 1], F32)
            nc.scalar.activation(actw[:], zc[:], AF.Identity, bias=zc[:])
            nc.scalar.activation(actw[:], zc[:], AF.Relu, bias=zc[:])
            nc.scalar.activation(actw[:], zc[:], AF.Exp, bias=zc[:])
            nc.scalar.copy(actw[:], zc[:])

            # ---- loads: transfers on one queue run sequentially but stripe
            # across the 16 HW DMA engines; per-queue order = priority ----
            # conv bias FIRST (tiny): every conv eviction needs it
            bct = const.tile([128, 8], F32)
            nc.scalar.dma_start(bct[:], bconv[:, :])
            b1s = const.tile([128, 2], F32)
            nc.scalar.dma_start(b1s[:], b1c[:, :])

            xloc = const.tile([128, HW2], BF16)
            wt = []
            for t in range(8):
                w = const.tile([128, 9 * 128], BF16, tag=f"wt{t}")
                wt.append(w)
            xa = const.tile([128, B * HW2], BF16)

            def load_wt(t, split=False):
                if split:
                    # 3-tap pieces: the first conv row-group can start after
                    # ~1/3 of wt0 plus xi chunk 0 has landed
                    for p0, p1 in ((0, 3), (3, 6), (6, 9)):
                        nc.gpsimd.dma_start(
                            wt[t][:, p0 * 128 : p1 * 128],
                            wconv[:, (t * 9 + p0) * 128 : (t * 9 + p1) * 128],
                        )
                else:
                    nc.gpsimd.dma_start(
                        wt[t][:], wconv[:, t * 9 * 128 : (t + 1) * 9 * 128]
                    )

            def load_slice_quad(s):
                # four adjacent slices = one contiguous 2.36MB transfer
                nc.gpsimd.dma_start(
                    xa[:, s * HW2 : (s + 4) * HW2],
                    xall[:, s * HW2 : (s + 4) * HW2],
                )

            # all loads on ONE gpsimd queue: per-queue FIFO order IS the
            # priority, so nothing big can steal ring bandwidth from conv
            # tile 0's inputs (split queues measurably starve the stream).
            nc.gpsimd.dma_start(xloc[:, 0 : 11 * HW], xi[:, 0 : 11 * HW])
            load_wt(0, split=True)
            for (rs, re) in XCHUNKS[1:]:
                nc.gpsimd.dma_start(
                    xloc[:, rs * HW : re * HW], xi[:, rs * HW : re * HW]
                )
            load_wt(1)
            load_wt(2)
            load_slice_quad(0)
            load_wt(3)
            load_slice_quad(4)
            load_wt(4)
            load_wt(5)
            load_wt(6)
            load_wt(7)

            # MLP consts on the scalar queue (concurrent, tiny)
            w1s = const.tile([128, HID], BF16)
            nc.scalar.dma_start(w1s[:], w1t[:, :])
            w2s = const.tile([128, 2 * K], BF16)
            nc.scalar.dma_start(w2s[:], w2t[:, :])
            b2s = const.tile([1, K], BF16)
            nc.scalar.dma_start(b2s[:], b2r[:, :])
            ones = const.tile([1, B], BF16)
            nc.scalar.dma_start(ones[:], one18[:, :])
            jreps = const.tile([B, 128], BF16)
            nc.scalar.dma_start(jreps[:], jrep[:, :])
            kexps = const.tile([K, 128], BF16)
            nc.scalar.dma_start(kexps[:], kexp[:, :])
            dmasks = const.tile([128, 128], BF16)
            nc.scalar.dma_start(dmasks[:], dmask[:, :])

            # ---- HAM pre-warm: dummy matmuls on a zeroed tile while the
            # first weights/xi DMA is in flight, so the real conv stream
            # starts at 2.4 GHz instead of paying the 1.2 GHz cold window ----
            zw = const.tile([128, 512], BF16)
            nc.vector.memset(zw[:], 0.0)
            # single reused psum tile: re-allocating per matmul adds a
            # tile-release sem round-trip between dummies (measured 2.8us gap)
            wp_ = psM.tile([128, 512], F32, tag="mlp")
            for _ in range(6):
                nc.tensor.matmul(
                    wp_[:], lhsT=zw[:, 0:128], rhs=zw[:], start=True, stop=True
                )

            # padded image built on-chip, one row-chunk at a time so conv
            # row-group g only waits for chunk g's DMA
            xp = const.tile([128, WP * WP], BF16)
            xp3 = xp[:].rearrange("p (h w) -> p h w", w=WP)
            nc.vector.memset(xp3[:, 0, 0:WP], 0.0)
            nc.vector.memset(xp3[:, WP - 1, 0:WP], 0.0)
            nc.vector.memset(xp3[:, 1 : 1 + HW, 0], 0.0)
            nc.vector.memset(xp3[:, 1 : 1 + HW, WP - 1], 0.0)
            xl3 = xloc[:].rearrange("p (h w) -> p h w", w=HW)
            for (rs, re) in XCHUNKS:
                nc.vector.tensor_copy(
                    xp3[:, 1 + rs : 1 + re, 1 : 1 + HW], xl3[:, rs:re, :]
                )

            # blend-weight scaffold: BD2[c*4+k, j*16+c] = attn[j, k], built
            # via two tiny matmuls + a mask multiply (all compute-engine ops
            # with plain APs — fully dependency-tracked, unlike the strided
            # DMA scatters this replaces).  Rows 64-127 replicate rows 0-63
            # (the mask pattern repeats) so the u=1 lhsT shares its base
            # partition with the partition-sliced rhs cs[64:128].
            BD2 = const.tile([128, 128], BF16)

            # ---- mean-pool all 8 samples (split DVE / ACT-accum so the
            # attention chain finishes ~8us earlier than all-DVE) ----
            poolf = const.tile([128, B], F32)
            nc.vector.memset(poolf[:], 0.0)
            ptrash = const.tile([128, HW2], BF16)
            for s in range(B):
                if s in (2, 6):
                    nc.scalar.activation(
                        ptrash[:],
                        xa[:, s * HW2 : (s + 1) * HW2],
                        AF.Identity,
                        accum_out=poolf[:, s : s + 1],
                    )
                else:
                    nc.vector.tensor_reduce(
                        poolf[:, s : s + 1],
                        xa[:, s * HW2 : (s + 1) * HW2],
                        axis=AX.X,
                        op=ALU.add,
                    )
            pooled8 = const.tile([128, B], BF16)
            nc.vector.tensor_copy(pooled8[:], poolf[:])

            cs_tiles = [None] * 8
            ob_tiles = {}
            store_cnt = [0]

            def store_half(g, c0, c1, ob, late=False, final=False):
                # late-tile stores stay OFF the gpsimd queue so its
                # end-of-kernel DMA drain (the tail critical path) is empty;
                # the very last small pieces go on scalar AFTER ACT's final
                # evictions, completing in parallel with sync's.
                if final:
                    eng = nc.scalar
                elif late:
                    eng = nc.sync
                else:
                    eng = [nc.gpsimd, nc.sync][store_cnt[0] % 2]
                    store_cnt[0] += 1
                eng.dma_start(out[:, 16 * g : 16 * g + 16, c0:c1], ob[:, c0:c1])

            def emit_pair(t, ci):
                """One (u=0, u=1) blend-matmul pair for chunk ci of tile t.
                The two matmuls occupy disjoint PE row strips and run
                concurrently."""
                cs = cs_tiles[t]
                c0, C = CHUNKS[ci]
                if ci == 0:
                    ob_u0 = osb_pool.tile([128, HW2], BF16, tag="osb")
                    ob_u1 = osb_pool.tile([128, HW2], BF16, tag="osb")
                    ob_tiles[t] = [ob_u0, ob_u1]
                for u in range(2):
                    ob = ob_tiles[t][u]
                    bp = psB.tile([128, C], F32, tag="bps")
                    nc.tensor.matmul(
                        bp[:],
                        lhsT=BD2[64 * u : 64 * u + 64, :],
                        rhs=cs[64 * u : 64 * u + 64, c0 : c0 + C],
                        start=True,
                        stop=True,
                    )
                    if u == 0:
                        nc.vector.tensor_copy(ob[:, c0 : c0 + C], bp[:])
                    else:
                        nc.scalar.copy(ob[:, c0 : c0 + C], bp[:])
                late = t >= 6
                if ci == 1:
                    for u in range(2):
                        store_half(2 * t + u, 0, 1024, ob_tiles[t][u], late)
                elif ci == 3:
                    for u in range(2):
                        store_half(2 * t + u, 1024, 2048, ob_tiles[t][u], late)
                elif ci == 4:
                    for u in range(2):
                        store_half(2 * t + u, 2048, HW2, ob_tiles[t][u],
                                   late, final=late)

            def emit_rg(t, gi):
                """One conv row-group: 9 shifted matmuls + biased eviction.
                Late tiles evict on DVE so ACT stays free for the trailing
                blend-pair evictions the last pairs wait on."""
                r0, R = ROW_GROUPS[gi]
                pt = psA.tile([128, R * HW], F32, tag="cps")
                for tap in range(9):
                    dh, dw = divmod(tap, 3)
                    rhs = xp3[:, r0 + dh : r0 + dh + R, dw : dw + HW]
                    nc.tensor.matmul(
                        pt[:],
                        lhsT=wt[t][:, tap * 128 : (tap + 1) * 128],
                        rhs=rhs,
                        start=(tap == 0),
                        stop=(tap == 8),
                    )
                dst = cs_tiles[t][:, r0 * HW : (r0 + R) * HW]
                nc.scalar.activation(
                    dst, pt[:], AF.Identity, bias=bct[:, t : t + 1]
                )

            def emit_conv(t, pairs=()):
                """Conv tile t with blend pairs interleaved after row-groups.
                pairs = list of (src_tile, chunk) per row-group slot (up to 2
                pairs per slot keeps psB within 4 banks)."""
                cs = csb_pool.tile([128, HW2], BF16, tag="csb")
                cs_tiles[t] = cs
                for gi in range(5):
                    emit_rg(t, gi)
                    for (pt_, ci_) in pairs[gi] if gi < len(pairs) else ():
                        # chunk ci reads cs columns < 512*(ci+1); row-groups
                        # 0..gi have evicted 480*(gi+1).  A same-tile blend
                        # emitted too early reads garbage (Tile deps only
                        # look backward in emission order).
                        assert pt_ != t or 480 * (gi + 1) >= 512 * (ci_ + 1), (
                            f"blend {pt_} chunk {ci_} hosted before its cs "
                            f"columns are evicted (tile {t} RG {gi})"
                        )
                        emit_pair(pt_, ci_)

            def emit_mlp():
                # attention MLP + double softmax for all 8 samples at once
                hd = []
                for h in range(2):
                    hps = psM.tile([128, B], F32, tag="mlp")
                    nc.tensor.matmul(
                        hps[:],
                        lhsT=w1s[:, h * 128 : (h + 1) * 128],
                        rhs=pooled8[:],
                        start=True,
                        stop=True,
                    )
                    hsb = const.tile([128, B], BF16, tag=f"hd{h}")
                    nc.scalar.activation(
                        hsb[:], hps[:], AF.Relu, bias=b1s[:, h : h + 1]
                    )
                    hd.append(hsb)

                lps = psM.tile([B, K], F32, tag="mlp")
                nc.tensor.matmul(
                    lps[:], lhsT=hd[0][:], rhs=w2s[:, 0:K], start=True, stop=False
                )
                nc.tensor.matmul(
                    lps[:], lhsT=hd[1][:], rhs=w2s[:, K : 2 * K],
                    start=False, stop=False,
                )
                nc.tensor.matmul(
                    lps[:], lhsT=ones[:], rhs=b2s[:], start=False, stop=True
                )

                # double softmax over k (shift-invariant: max-sub dropped)
                e1 = const.tile([B, K], F32)
                nc.scalar.activation(e1[:], lps[:], AF.Exp, bias=0.0, scale=1.0)
                s1 = const.tile([B, 1], F32)
                nc.vector.tensor_reduce(s1[:], e1[:], axis=AX.X, op=ALU.add)
                r1 = const.tile([B, 1], F32)
                nc.vector.reciprocal(r1[:], s1[:])
                a1 = const.tile([B, K], F32)
                nc.vector.tensor_scalar_mul(a1[:], e1[:], r1[:, 0:1])

                e2 = const.tile([B, K], F32)
                nc.scalar.activation(
                    e2[:], a1[:], AF.Exp, bias=0.0, scale=1.0 / TAU
                )
                s2 = const.tile([B, 1], F32)
                nc.vector.tensor_reduce(s2[:], e2[:], axis=AX.X, op=ALU.add)
                r2 = const.tile([B, 1], F32)
                nc.vector.reciprocal(r2[:], s2[:])
                attn = const.tile([B, K], BF16)
                nc.vector.tensor_scalar_mul(attn[:], e2[:], r2[:, 0:1])

                # blend weights without DMA scatters:
                #   R[k, col]     = sum_j attn[j,k] * jrep[j, col]
                #                 = attn[col//16, k]
                #   BDfull[p,col] = sum_k kexp[k,p] * R[k, col]
                #                 = attn[col//16, p%4]
                #   BD2           = BDfull * dmask   (zero off the c-diagonal)
                rps = psM.tile([K, 128], F32, tag="mlp")
                nc.tensor.matmul(
                    rps[:], lhsT=attn[:], rhs=jreps[:], start=True, stop=True
                )
                rsb = const.tile([K, 128], BF16, tag="rsb")
                nc.scalar.copy(rsb[:], rps[:])
                bfull = psM.tile([128, 128], F32, tag="mlp")
                nc.tensor.matmul(
                    bfull[:], lhsT=kexps[:], rhs=rsb[:], start=True, stop=True
                )
                nc.vector.tensor_tensor(
                    BD2[:], bfull[:], dmasks[:], op=ALU.mult
                )

            # ---- PE stream ----
            emit_conv(0)
            emit_conv(1)
            emit_mlp()
            emit_conv(2)
            emit_conv(3, pairs=[
                [(0, 0), (1, 0)], [(0, 1), (1, 1)], [(0, 2), (1, 2)],
                [(0, 3), (1, 3)], [(0, 4), (1, 4)],
            ])
            emit_conv(4, pairs=[
                [(2, 0), (3, 0)], [(2, 1), (3, 1)], [(2, 2), (3, 2)],
                [(2, 3), (3, 3)], [(2, 4), (3, 4)],
            ])
            emit_conv(5, pairs=[
                [(4, 0)], [(4, 1), (5, 0)], [(4, 2), (5, 1)],
                [(4, 3), (5, 2)], [(4, 4), (5, 3)],
            ])
            emit_pair(5, 4)
            # blends 6 and 7 each ride their own conv tile, staggered: chunk
            # ci only needs cs columns evicted through row-group ci+1 (a
            # blend must never read cs regions whose evictions are emitted
            # later — Tile deps only look backward), so pairs trail one
            # group behind and only the small chunk 4 drains after the tile.
            emit_conv(6, pairs=[
                [], [(6, 0)], [(6, 1)], [(6, 2)], [(6, 3)],
            ])
            emit_pair(6, 4)
            emit_conv(7, pairs=[
                [], [(7, 0)], [(7, 1)], [(7, 2)], [(7, 3)],
            ])
            emit_pair(7, 4)

    nc.compile()
    return nc


def pack_inputs(x, conv_w, conv_b, w1, b1, w2, b2):
    """Host-side layout packing + bf16 casts (the mean-pool 1/HW^2 scale is
    folded into w1)."""
    bf = ml_dtypes.bfloat16
    x = np.ascontiguousarray(x, dtype=np.float32)
    x_all = x.reshape(B, CIN, HW2).astype(bf)
    xall_T = np.ascontiguousarray(
        x_all.transpose(1, 0, 2).reshape(CIN, B * HW2)
    )

    # conv_w [K, COUT, CIN, 3, 3] -> [ci, t, tap, p] with p = c*4 + k,
    # co = 32 t + c
    w = np.asarray(conv_w, dtype=np.float32).transpose(2, 3, 4, 0, 1)  # ci kh kw k co
    w = w.reshape(CIN, KS, KS, K, 8, 32)  # ci kh kw k t c
    w = w.transpose(0, 4, 1, 2, 5, 3)  # ci t kh kw c k
    wconv = np.ascontiguousarray(w.reshape(CIN, 8 * 9 * 128)).astype(bf)

    bc = np.asarray(conv_b, dtype=np.float32).reshape(K, 8, 32)  # k t c
    bconv = np.ascontiguousarray(bc.transpose(1, 2, 0).reshape(8, 128).T)  # [p, t]

    w1t = (np.ascontiguousarray(np.asarray(w1, dtype=np.float32).T) / float(HW2)).astype(bf)
    b1c = np.ascontiguousarray(np.asarray(b1, dtype=np.float32).reshape(2, 128).T)
    w2T = np.asarray(w2, dtype=np.float32).T  # [256, 4]
    w2t = np.ascontiguousarray(
        np.concatenate([w2T[:128], w2T[128:]], axis=1)
    ).astype(bf)
    b2r = np.asarray(b2, dtype=np.float32).reshape(1, K).astype(bf)

    cols = np.arange(128)
    parts = np.arange(128)
    jrep = (cols[None, :] // 16 == np.arange(B)[:, None]).astype(np.float32)
    kexp = (parts[None, :] % 4 == np.arange(K)[:, None]).astype(np.float32)
    dmask = (cols[None, :] % 16 == (parts[:, None] // 4) % 16).astype(np.float32)

    common = dict(
        xall=xall_T, wconv=wconv, bconv=bconv, w1t=w1t, b1c=b1c,
        w2t=w2t, b2r=b2r,
        one18=np.ones((1, B), dtype=np.float32).astype(bf),
        jrep=jrep.astype(bf), kexp=kexp.astype(bf), dmask=dmask.astype(bf),
    )
    in_maps = [
        dict(common, xi=np.ascontiguousarray(x_all[i])) for i in range(NCORES)
    ]
    return in_maps


def run(inputs, trace=False):
    from concourse.bass_utils import run_bass_kernel_spmd

    nc = build_nc()
    in_maps = pack_inputs(**inputs)
    res = run_bass_kernel_spmd(
        nc, in_maps, core_ids=list(range(NCORES)), trace=trace
    )
    slabs = [
        np.asarray(res.results[i]["out"]).astype(np.float32)
        for i in range(NCORES)
    ]
    out = np.stack(slabs, axis=0).reshape(B, B, COUT, HW, HW)
    return out, res


def kernel(**inputs) -> np.ndarray:
    out, _ = run(inputs, trace=False)
    return out


# revision 48
# speedup vs baseline: 1.0146x; 1.0146x over previous
"""Trainium2 Bass kernel for nn_DynamicConv (dense_cnn).

out[i, j, co, h, w] = sum_k (conv_k(x_i)[co, h, w] + b_k[co]) * attn[j, k]
attn = softmax(softmax(MLP(meanpool(x)), k) / TAU, k)

Sharding: data-parallel over batch i across 8 cores.  Each core convolves its
own sample (9 shifted bf16 matmuls over a zero-padded image, contraction =
CIN=128) and computes the full [B, K] attention matrix locally: every core
loads all 8 x-slices (bf16), mean-pools them on DVE, and runs the tiny MLP +
double softmax itself — no collective at all.

The cross-batch blend is a block-diagonal bf16 matmul per 16-channel group
(contraction 64 in one partition half, M = 128 = j8 x co16).  The two halves
(u=0 partitions 0-63, u=1 partitions 64-127) are emitted as ADJACENT matmuls:
they land on disjoint PE row-group strips (tile_position (0,0) / (64,0)) and
execute CONCURRENTLY.  One pair is interleaved after each conv row-group so
PSUM evictions never gate the PE: the pair's two banks drain on DVE+ACT well
within the next row-group's ~2us of conv matmuls.

Startup: xi is loaded in 5 row-chunks so the first conv matmul only waits for
~130KB + wt0; a short burst of dummy matmuls during the DMA wait warms the PE
HAM clock-gate so the real stream starts at 2.4 GHz.  Output slabs are stored
in halves as soon as their chunks evict, spread across the gpsimd and sync
DMA queues, shrinking the end-of-kernel DMA drain.

All matmul operands are bf16 (PE full rate); PSUM accumulates fp32; the
output slab is stored bf16 and widened to fp32 on the host.
"""

import sys

import numpy as np

if "/opt/trn_rl_repo" not in sys.path:
    sys.path.insert(0, "/opt/trn_rl_repo")

import ml_dtypes

import concourse.bacc as bacc
import concourse.bass as bass
import concourse.mybir as mybir
import concourse.tile as tile

F32 = mybir.dt.float32
BF16 = mybir.dt.bfloat16
AF = mybir.ActivationFunctionType
AX = mybir.AxisListType
ALU = mybir.AluOpType

B = 8
CIN = 128
COUT = 256
K = 4
KS = 3
HW = 48
HW2 = HW * HW          # 2304
WP = HW + 2            # 50 (padded)
HID = 256
TAU = 30.0
NCORES = 8

ROW_GROUPS = [(0, 10), (10, 10), (20, 10), (30, 10), (40, 8)]
CHUNKS = [(0, 512), (512, 512), (1024, 512), (1536, 512), (2048, 256)]
# xi row-chunk boundaries: chunk c covers exactly what conv row-group c needs
XCHUNKS = [(0, 11), (11, 21), (21, 31), (31, 41), (41, 48)]


def build_nc():
    nc = bacc.Bacc("TRN2", debug=False, num_devices=NCORES)

    xi = nc.dram_tensor("xi", [CIN, HW2], BF16, kind="ExternalInput").ap()
    xall = nc.dram_tensor("xall", [CIN, B * HW2], BF16, kind="ExternalInput").ap()
    # [ci, t, tap, p] flattened; p = c*4 + k encodes (co = 32 t + c, k)
    wconv = nc.dram_tensor(
        "wconv", [CIN, 8 * 9 * 128], BF16, kind="ExternalInput"
    ).ap()
    bconv = nc.dram_tensor("bconv", [128, 8], F32, kind="ExternalInput").ap()
    w1t = nc.dram_tensor("w1t", [CIN, HID], BF16, kind="ExternalInput").ap()
    b1c = nc.dram_tensor("b1c", [128, 2], F32, kind="ExternalInput").ap()
    w2t = nc.dram_tensor("w2t", [128, 2 * K], BF16, kind="ExternalInput").ap()
    b2r = nc.dram_tensor("b2r", [1, K], BF16, kind="ExternalInput").ap()
    one18 = nc.dram_tensor("one18", [1, B], BF16, kind="ExternalInput").ap()
    # constant selectors for the DMA-free BD2 build:
    #   jrep[j, col] = (col // 16 == j);  kexp[k, p] = (p % 4 == k)
    #   dmask[p, col] = (col % 16 == (p // 4) % 16)
    jrep = nc.dram_tensor("jrep", [B, 128], BF16, kind="ExternalInput").ap()
    kexp = nc.dram_tensor("kexp", [K, 128], BF16, kind="ExternalInput").ap()
    dmask = nc.dram_tensor("dmask", [128, 128], BF16, kind="ExternalInput").ap()
    out = nc.dram_tensor("out", [B, COUT, HW2], BF16, kind="ExternalOutput").ap()

    with tile.TileContext(nc, num_cores=NCORES) as tc:
        with (
            tc.tile_pool(name="const", bufs=1) as const,
            tc.tile_pool(name="csb", bufs=8) as csb_pool,
            tc.tile_pool(name="osb", bufs=8) as osb_pool,
            tc.tile_pool(name="psA", bufs=3, space="PSUM") as psA,
            tc.tile_pool(name="psB", bufs=4, space="PSUM") as psB,
            tc.tile_pool(name="psM", bufs=1, space="PSUM") as psM,
        ):
            # pre-warm the ACT function tables (1.3us each if loaded lazily
            # inside the latency-critical chains)
            zc = const.tile([128, 1], F32)
            nc.gpsimd.memset(zc[:], 0.0)
            actw = const.tile([128, 1], F32)
            nc.scalar.activation(actw[:], zc[:], AF.Identity, bias=zc[:])
            nc.scalar.activation(actw[:], zc[:], AF.Relu, bias=zc[:])
            nc.scalar.activation(actw[:], zc[:], AF.Exp, bias=zc[:])
            nc.scalar.copy(actw[:], zc[:])

            # ---- loads: transfers on one queue run sequentially but stripe
            # across the 16 HW DMA engines; per-queue order = priority ----
            # conv bias FIRST (tiny): every conv eviction needs it
            bct = const.tile([128, 8], F32)
            nc.scalar.dma_start(bct[:], bconv[:, :])
            b1s = const.tile([128, 2], F32)
            nc.scalar.dma_start(b1s[:], b1c[:, :])

            xloc = const.tile([128, HW2], BF16)
            wt = []
            for t in range(8):
                w = const.tile([128, 9 * 128], BF16, tag=f"wt{t}")
                wt.append(w)
            xa = const.tile([128, B * HW2], BF16)

            def load_wt(t, split=False):
                if split:
                    # 3-tap pieces: the first conv row-group can start after
                    # ~1/3 of wt0 plus xi chunk 0 has landed
                    for p0, p1 in ((0, 3), (3, 6), (6, 9)):
                        nc.gpsimd.dma_start(
                            wt[t][:, p0 * 128 : p1 * 128],
                            wconv[:, (t * 9 + p0) * 128 : (t * 9 + p1) * 128],
                        )
                else:
                    nc.gpsimd.dma_start(
                        wt[t][:], wconv[:, t * 9 * 128 : (t + 1) * 9 * 128]
                    )

            def load_slice_quad(s):
                # four adjacent slices = one contiguous 2.36MB transfer
                nc.gpsimd.dma_start(
                    xa[:, s * HW2 : (s + 4) * HW2],
                    xall[:, s * HW2 : (s + 4) * HW2],
                )

            # all loads on ONE gpsimd queue: per-queue FIFO order IS the
            # priority, so nothing big can steal ring bandwidth from conv
            # tile 0's inputs (split queues measurably starve the stream).
            nc.gpsimd.dma_start(xloc[:, 0 : 11 * HW], xi[:, 0 : 11 * HW])
            load_wt(0, split=True)
            for (rs, re) in XCHUNKS[1:]:
                nc.gpsimd.dma_start(
                    xloc[:, rs * HW : re * HW], xi[:, rs * HW : re * HW]
                )
            load_wt(1)
            load_wt(2)
            load_slice_quad(0)
            load_wt(3)
            load_slice_quad(4)
            load_wt(4)
            load_wt(5)
            load_wt(6)
            load_wt(7)

            # MLP consts on the scalar queue (concurrent, tiny)
            w1s = const.tile([128, HID], BF16)
            nc.scalar.dma_start(w1s[:], w1t[:, :])
            w2s = const.tile([128, 2 * K], BF16)
            nc.scalar.dma_start(w2s[:], w2t[:, :])
            b2s = const.tile([1, K], BF16)
            nc.scalar.dma_start(b2s[:], b2r[:, :])
            ones = const.tile([1, B], BF16)
            nc.scalar.dma_start(ones[:], one18[:, :])
            jreps = const.tile([B, 128], BF16)
            nc.scalar.dma_start(jreps[:], jrep[:, :])
            kexps = const.tile([K, 128], BF16)
            nc.scalar.dma_start(kexps[:], kexp[:, :])
            dmasks = const.tile([128, 128], BF16)
            nc.scalar.dma_start(dmasks[:], dmask[:, :])

            # ---- HAM pre-warm: dummy matmuls on a zeroed tile while the
            # first weights/xi DMA is in flight, so the real conv stream
            # starts at 2.4 GHz instead of paying the 1.2 GHz cold window ----
            zw = const.tile([128, 512], BF16)
            nc.vector.memset(zw[:], 0.0)
            # single reused psum tile: re-allocating per matmul adds a
            # tile-release sem round-trip between dummies (measured 2.8us gap)
            wp_ = psM.tile([128, 512], F32, tag="mlp")
            for _ in range(4):
                nc.tensor.matmul(
                    wp_[:], lhsT=zw[:, 0:128], rhs=zw[:], start=True, stop=True
                )

            # padded image built on-chip, one row-chunk at a time so conv
            # row-group g only waits for chunk g's DMA
            xp = const.tile([128, WP * WP], BF16)
            xp3 = xp[:].rearrange("p (h w) -> p h w", w=WP)
            nc.vector.memset(xp3[:, 0, 0:WP], 0.0)
            nc.vector.memset(xp3[:, WP - 1, 0:WP], 0.0)
            nc.vector.memset(xp3[:, 1 : 1 + HW, 0], 0.0)
            nc.vector.memset(xp3[:, 1 : 1 + HW, WP - 1], 0.0)
            xl3 = xloc[:].rearrange("p (h w) -> p h w", w=HW)
            for (rs, re) in XCHUNKS:
                nc.vector.tensor_copy(
                    xp3[:, 1 + rs : 1 + re, 1 : 1 + HW], xl3[:, rs:re, :]
                )

            # blend-weight scaffold: BD2[c*4+k, j*16+c] = attn[j, k], built
            # via two tiny matmuls + a mask multiply (all compute-engine ops
            # with plain APs — fully dependency-tracked, unlike the strided
            # DMA scatters this replaces).  Rows 64-127 replicate rows 0-63
            # (the mask pattern repeats) so the u=1 lhsT shares its base
            # partition with the partition-sliced rhs cs[64:128].
            BD2 = const.tile([128, 128], BF16)

            # ---- mean-pool all 8 samples (split DVE / ACT-accum so the
            # attention chain finishes ~8us earlier than all-DVE) ----
            poolf = const.tile([128, B], F32)
            nc.vector.memset(poolf[:], 0.0)
            ptrash = const.tile([128, HW2], BF16)
            for s in range(B):
                if s in (2, 6):
                    nc.scalar.activation(
                        ptrash[:],
                        xa[:, s * HW2 : (s + 1) * HW2],
                        AF.Identity,
                        accum_out=poolf[:, s : s + 1],
                    )
                else:
                    nc.vector.tensor_reduce(
                        poolf[:, s : s + 1],
                        xa[:, s * HW2 : (s + 1) * HW2],
                        axis=AX.X,
                        op=ALU.add,
                    )
            pooled8 = const.tile([128, B], BF16)
            nc.vector.tensor_copy(pooled8[:], poolf[:])

            cs_tiles = [None] * 8
            ob_tiles = {}
            store_cnt = [0]

            def store_half(g, c0, c1, ob, wide=False):
                # late stores fan out over four queues (ACT/DVE are idle by
                # then) so the final pieces issue and complete in parallel
                engs = [nc.gpsimd, nc.sync]
                eng = engs[store_cnt[0] % len(engs)]
                store_cnt[0] += 1
                eng.dma_start(out[:, 16 * g : 16 * g + 16, c0:c1], ob[:, c0:c1])

            def emit_pair(t, ci):
                """One (u=0, u=1) blend-matmul pair for chunk ci of tile t.
                The two matmuls occupy disjoint PE row strips and run
                concurrently."""
                cs = cs_tiles[t]
                c0, C = CHUNKS[ci]
                if ci == 0:
                    ob_u0 = osb_pool.tile([128, HW2], BF16, tag="osb")
                    ob_u1 = osb_pool.tile([128, HW2], BF16, tag="osb")
                    ob_tiles[t] = [ob_u0, ob_u1]
                for u in range(2):
                    ob = ob_tiles[t][u]
                    bp = psB.tile([128, C], F32, tag="bps")
                    nc.tensor.matmul(
                        bp[:],
                        lhsT=BD2[64 * u : 64 * u + 64, :],
                        rhs=cs[64 * u : 64 * u + 64, c0 : c0 + C],
                        start=True,
                        stop=True,
                    )
                    if t >= 6:
                        # conv evictions moved to DVE for these tiles
                        nc.scalar.copy(ob[:, c0 : c0 + C], bp[:])
                    elif u == 0:
                        nc.vector.tensor_copy(ob[:, c0 : c0 + C], bp[:])
                    else:
                        nc.scalar.copy(ob[:, c0 : c0 + C], bp[:])
                wide = t >= 6
                if ci == 1:
                    for u in range(2):
                        store_half(2 * t + u, 0, 1024, ob_tiles[t][u], wide)
                elif ci == 3:
                    for u in range(2):
                        store_half(2 * t + u, 1024, 2048, ob_tiles[t][u], wide)
                elif ci == 4:
                    for u in range(2):
                        store_half(2 * t + u, 2048, HW2, ob_tiles[t][u], wide)

            def emit_rg(t, gi):
                """One conv row-group: 9 shifted matmuls + biased eviction.
                Late tiles evict on DVE so ACT stays free for the trailing
                blend-pair evictions the last pairs wait on."""
                r0, R = ROW_GROUPS[gi]
                pt = psA.tile([128, R * HW], F32, tag="cps")
                for tap in range(9):
                    dh, dw = divmod(tap, 3)
                    rhs = xp3[:, r0 + dh : r0 + dh + R, dw : dw + HW]
                    nc.tensor.matmul(
                        pt[:],
                        lhsT=wt[t][:, tap * 128 : (tap + 1) * 128],
                        rhs=rhs,
                        start=(tap == 0),
                        stop=(tap == 8),
                    )
                dst = cs_tiles[t][:, r0 * HW : (r0 + R) * HW]
                if t >= 6:
                    nc.vector.tensor_scalar_add(dst, pt[:], bct[:, t : t + 1])
                else:
                    nc.scalar.activation(
                        dst, pt[:], AF.Identity, bias=bct[:, t : t + 1]
                    )

            def emit_conv(t, pairs=()):
                """Conv tile t with blend pairs interleaved after row-groups.
                pairs = list of (src_tile, chunk) per row-group slot (up to 2
                pairs per slot keeps psB within 4 banks)."""
                cs = csb_pool.tile([128, HW2], BF16, tag="csb")
                cs_tiles[t] = cs
                for gi in range(5):
                    emit_rg(t, gi)
                    for (pt_, ci_) in pairs[gi] if gi < len(pairs) else ():
                        # chunk ci reads cs columns < 512*(ci+1); row-groups
                        # 0..gi have evicted 480*(gi+1).  A same-tile blend
                        # emitted too early reads garbage (Tile deps only
                        # look backward in emission order).
                        assert pt_ != t or 480 * (gi + 1) >= 512 * (ci_ + 1), (
                            f"blend {pt_} chunk {ci_} hosted before its cs "
                            f"columns are evicted (tile {t} RG {gi})"
                        )
                        emit_pair(pt_, ci_)

            def emit_mlp():
                # attention MLP + double softmax for all 8 samples at once
                hd = []
                for h in range(2):
                    hps = psM.tile([128, B], F32, tag="mlp")
                    nc.tensor.matmul(
                        hps[:],
                        lhsT=w1s[:, h * 128 : (h + 1) * 128],
                        rhs=pooled8[:],
                        start=True,
                        stop=True,
                    )
                    hsb = const.tile([128, B], BF16, tag=f"hd{h}")
                    nc.scalar.activation(
                        hsb[:], hps[:], AF.Relu, bias=b1s[:, h : h + 1]
                    )
                    hd.append(hsb)

                lps = psM.tile([B, K], F32, tag="mlp")
                nc.tensor.matmul(
                    lps[:], lhsT=hd[0][:], rhs=w2s[:, 0:K], start=True, stop=False
                )
                nc.tensor.matmul(
                    lps[:], lhsT=hd[1][:], rhs=w2s[:, K : 2 * K],
                    start=False, stop=False,
                )
                nc.tensor.matmul(
                    lps[:], lhsT=ones[:], rhs=b2s[:], start=False, stop=True
                )

                # double softmax over k (shift-invariant: max-sub dropped)
                e1 = const.tile([B, K], F32)
                nc.scalar.activation(e1[:], lps[:], AF.Exp, bias=0.0, scale=1.0)
                s1 = const.tile([B, 1], F32)
                nc.vector.tensor_reduce(s1[:], e1[:], axis=AX.X, op=ALU.add)
                r1 = const.tile([B, 1], F32)
                nc.vector.reciprocal(r1[:], s1[:])
                a1 = const.tile([B, K], F32)
                nc.vector.tensor_scalar_mul(a1[:], e1[:], r1[:, 0:1])

                e2 = const.tile([B, K], F32)
                nc.scalar.activation(
                    e2[:], a1[:], AF.Exp, bias=0.0, scale=1.0 / TAU
                )
                s2 = const.tile([B, 1], F32)
                nc.vector.tensor_reduce(s2[:], e2[:], axis=AX.X, op=ALU.add)
                r2 = const.tile([B, 1], F32)
                nc.vector.reciprocal(r2[:], s2[:])
                attn = const.tile([B, K], BF16)
                nc.vector.tensor_scalar_mul(attn[:], e2[:], r2[:, 0:1])

                # blend weights without DMA scatters:
                #   R[k, col]     = sum_j attn[j,k] * jrep[j, col]
                #                 = attn[col//16, k]
                #   BDfull[p,col] = sum_k kexp[k,p] * R[k, col]
                #                 = attn[col//16, p%4]
                #   BD2           = BDfull * dmask   (zero off the c-diagonal)
                rps = psM.tile([K, 128], F32, tag="mlp")
                nc.tensor.matmul(
                    rps[:], lhsT=attn[:], rhs=jreps[:], start=True, stop=True
                )
                rsb = const.tile([K, 128], BF16, tag="rsb")
                nc.scalar.copy(rsb[:], rps[:])
                bfull = psM.tile([128, 128], F32, tag="mlp")
                nc.tensor.matmul(
                    bfull[:], lhsT=kexps[:], rhs=rsb[:], start=True, stop=True
                )
                nc.vector.tensor_tensor(
                    BD2[:], bfull[:], dmasks[:], op=ALU.mult
                )

            # ---- PE stream ----
            emit_conv(0)
            emit_conv(1)
            emit_mlp()
            emit_conv(2)
            emit_conv(3, pairs=[
                [(0, 0), (1, 0)], [(0, 1), (1, 1)], [(0, 2), (1, 2)],
                [(0, 3), (1, 3)], [(0, 4), (1, 4)],
            ])
            emit_conv(4, pairs=[
                [(2, 0), (3, 0)], [(2, 1), (3, 1)], [(2, 2), (3, 2)],
                [(2, 3), (3, 3)], [(2, 4), (3, 4)],
            ])
            emit_conv(5, pairs=[
                [(4, 0)], [(4, 1), (5, 0)], [(4, 2), (5, 1)],
                [(4, 3), (5, 2)], [(4, 4), (5, 3)],
            ])
            emit_pair(5, 4)
            # blends 6 and 7 each ride their own conv tile, staggered: chunk
            # ci only needs cs columns evicted through row-group ci+1 (a
            # blend must never read cs regions whose evictions are emitted
            # later — Tile deps only look backward), so pairs trail one
            # group behind and only the small chunk 4 drains after the tile.
            emit_conv(6, pairs=[
                [], [(6, 0)], [(6, 1)], [(6, 2)], [(6, 3)],
            ])
            emit_pair(6, 4)
            emit_conv(7, pairs=[
                [], [(7, 0)], [(7, 1)], [(7, 2)], [(7, 3)],
            ])
            emit_pair(7, 4)

    nc.compile()
    return nc


def pack_inputs(x, conv_w, conv_b, w1, b1, w2, b2):
    """Host-side layout packing + bf16 casts (the mean-pool 1/HW^2 scale is
    folded into w1)."""
    bf = ml_dtypes.bfloat16
    x = np.ascontiguousarray(x, dtype=np.float32)
    x_all = x.reshape(B, CIN, HW2).astype(bf)
    xall_T = np.ascontiguousarray(
        x_all.transpose(1, 0, 2).reshape(CIN, B * HW2)
    )

    # conv_w [K, COUT, CIN, 3, 3] -> [ci, t, tap, p] with p = c*4 + k,
    # co = 32 t + c
    w = np.asarray(conv_w, dtype=np.float32).transpose(2, 3, 4, 0, 1)  # ci kh kw k co
    w = w.reshape(CIN, KS, KS, K, 8, 32)  # ci kh kw k t c
    w = w.transpose(0, 4, 1, 2, 5, 3)  # ci t kh kw c k
    wconv = np.ascontiguousarray(w.reshape(CIN, 8 * 9 * 128)).astype(bf)

    bc = np.asarray(conv_b, dtype=np.float32).reshape(K, 8, 32)  # k t c
    bconv = np.ascontiguousarray(bc.transpose(1, 2, 0).reshape(8, 128).T)  # [p, t]

    w1t = (np.ascontiguousarray(np.asarray(w1, dtype=np.float32).T) / float(HW2)).astype(bf)
    b1c = np.ascontiguousarray(np.asarray(b1, dtype=np.float32).reshape(2, 128).T)
    w2T = np.asarray(w2, dtype=np.float32).T  # [256, 4]
    w2t = np.ascontiguousarray(
        np.concatenate([w2T[:128], w2T[128:]], axis=1)
    ).astype(bf)
    b2r = np.asarray(b2, dtype=np.float32).reshape(1, K).astype(bf)

    cols = np.arange(128)
    parts = np.arange(128)
    jrep = (cols[None, :] // 16 == np.arange(B)[:, None]).astype(np.float32)
    kexp = (parts[None, :] % 4 == np.arange(K)[:, None]).astype(np.float32)
    dmask = (cols[None, :] % 16 == (parts[:, None] // 4) % 16).astype(np.float32)

    common = dict(
        xall=xall_T, wconv=wconv, bconv=bconv, w1t=w1t, b1c=b1c,
        w2t=w2t, b2r=b2r,
        one18=np.ones((1, B), dtype=np.float32).astype(bf),
        jrep=jrep.astype(bf), kexp=kexp.astype(bf), dmask=dmask.astype(bf),
    )
    in_maps = [
        dict(common, xi=np.ascontiguousarray(x_all[i])) for i in range(NCORES)
    ]
    return in_maps


def run(inputs, trace=False):
    from concourse.bass_utils import run_bass_kernel_spmd

    nc = build_nc()
    in_maps = pack_inputs(**inputs)
    res = run_bass_kernel_spmd(
        nc, in_maps, core_ids=list(range(NCORES)), trace=trace
    )
    slabs = [
        np.asarray(res.results[i]["out"]).astype(np.float32)
        for i in range(NCORES)
    ]
    out = np.stack(slabs, axis=0).reshape(B, B, COUT, HW, HW)
    return out, res


def kernel(**inputs) -> np.ndarray:
    out, _ = run(inputs, trace=False)
    return out
